# revision 1
# baseline (speedup 1.0000x reference)
"""Causal self-attention (B=4, T=2048, C=2048, H=16, RoPE) on 8 trn2 cores.

Sharding: core c -> (batch b = c//2, head-group g = c%2), 8 heads per core.
Each core computes y_partial[b] = attn_heads(g) @ W_proj[rows(g)]; the host
sums the two partials per batch.

v2 design (single fused TileContext, no phase barriers, no DRAM scratch for
q/k/v/o):
  - All matmul inputs are bf16 (1 cycle/row on the PE, same as f32r, but
    halves SBUF + DMA). PSUM accumulation stays f32, output y is f32.
  - Heads are processed in pairs; QKV projection+RoPE for pair p is emitted
    interleaved with attention for pair p-1 so the PE stream always has
    independent matmuls to chew on while ScalarE exp / Pool mask / DVE
    rowsum chains drain (PE is in-order per engine; gaps also drop it out
    of its 2.4GHz p-state, so gap-free emission is ~2x).
  - RoPE: pair-swap via PE matmul on a +-1 permutation, then two DVE muls
    (cos/sin) and an add that writes the resident bf16 qT/kT tiles.
  - Softmax: exp on ScalarE straight from PSUM scores; causal masking only
    on the 4 diagonal blocks via Pool (gpsimd) mul; row-sums accumulated in
    f32 alternating DVE/Pool, reduced across partitions by a ones-matmul,
    reciprocal on DVE, broadcast via a tiny DRAM round-trip, applied to
    O^T from PSUM. AV matmuls are emitted with lag-2 behind the score
    matmuls so exp latency never stalls the PE.
  - O^T stays resident in SBUF (bf16) and feeds the output projection.
PSUM budget: qk2 + swap1 + v1 + scores2(+rowsum shared) + o2 = 8 banks.
"""
import sys

sys.path.insert(0, "/opt/trn_rl_repo")

import numpy as np
import ml_dtypes

BF16 = ml_dtypes.bfloat16

B, T, C, H, D = 4, 2048, 2048, 16, 128
G = 2                      # head groups (tensor-parallel dim)
HG = H // G                # heads per core = 8
CG = HG * D                # channels per group = 1024
P = 128
NQ = T // 512              # q chunks of 512
KO = C // P                # contraction chunks = 16
NPAIR = HG // 2            # head pairs per core = 4
ROPE_BASE = 10000.0
SCALE = 1.0 / float(np.sqrt(D))
N_CORES = 8

_cached = None


def _build_program(reps=1, phases="all", variant="full", bench_mode=False,
                   zip_pairs=False):
    import concourse.bass as bass
    import concourse.tile as tile
    from concourse import bacc, mybir

    f32 = mybir.dt.float32
    f32r = mybir.dt.float32r
    bf16 = mybir.dt.bfloat16
    Exp = mybir.ActivationFunctionType.Exp

    nc = bacc.Bacc()

    # host-prepacked bf16 inputs: leading axis indexes a chunk, then
    # [partition, ko, free] with long contiguous rows.
    xq_d = nc.declare_dram_parameter("xq", [4, P, KO, 512], bf16, isOutput=False)
    wq_d = nc.declare_dram_parameter("wq", [HG, P, KO, D], bf16, isOutput=False)
    wk_d = nc.declare_dram_parameter("wk", [HG, P, KO, D], bf16, isOutput=False)
    wv_d = nc.declare_dram_parameter("wv", [NPAIR, P, KO, 256], bf16, isOutput=False)
    wp_d = nc.declare_dram_parameter("wp", [4, P, HG, 512], bf16, isOutput=False)
    cos_d = nc.declare_dram_parameter("cosT", [P, T], f32, isOutput=False)
    sin_d = nc.declare_dram_parameter("sinT", [P, T], f32, isOutput=False)
    swp_d = nc.declare_dram_parameter("swapT", [P, P], f32, isOutput=False)
    ones_d = nc.declare_dram_parameter("ones", [P, 1], bf16, isOutput=False)
    mask_d = nc.declare_dram_parameter("masks", [P, 4, 512], bf16, isOutput=False)
    ident_d = nc.declare_dram_parameter("ident", [P, P], bf16, isOutput=False)
    if bench_mode:
        # identical device work, but y goes to scratch and only a tiny token
        # is an ExternalOutput -> host transfer floor vanishes for timing
        y_d = nc.dram_tensor("y_scratch", [T, C], f32)
        tok_d = nc.declare_dram_parameter("tok", [P, P], f32, isOutput=True)
    else:
        y_d = nc.declare_dram_parameter("y", [T, C], f32, isOutput=True)
        tok_d = None

    inv_s = nc.dram_tensor("inv_s", [HG, NQ, 1, 512], f32)

    for _rep in range(reps):
        with tile.TileContext(nc) as tc:
            with tc.tile_pool(name="const", bufs=1) as cp, \
                 tc.tile_pool(name="oall", bufs=1) as oap, \
                 tc.tile_pool(name="qkres", bufs=2) as qkp, \
                 tc.tile_pool(name="vres", bufs=2) as vhp, \
                 tc.tile_pool(name="sm", bufs=2) as smp, \
                 tc.tile_pool(name="ptp", bufs=3) as ptp, \
                 tc.tile_pool(name="psA", bufs=2, space="PSUM") as psA, \
                 tc.tile_pool(name="psV", bufs=1, space="PSUM") as psV, \
                 tc.tile_pool(name="psS", bufs=2, space="PSUM") as psS, \
                 tc.tile_pool(name="psN", bufs=1, space="PSUM") as psN, \
                 tc.tile_pool(name="psO", bufs=1, space="PSUM") as psO:

                cosT = cp.tile([P, T], f32)
                sinT = cp.tile([P, T], f32)
                swpT = cp.tile([P, P], f32r)
                ones = cp.tile([P, 1], bf16)
                masks = cp.tile([P, 4, 512], bf16)
                ident = cp.tile([P, P], bf16)
                o_all = oap.tile([P, HG, T], bf16)
                # swpT on the sync queue (needed ~3.5us in, ahead of x/w);
                # the rest ride the gpsimd queue in parallel.
                nc.sync.dma_start(swpT[:], swp_d.ap().bitcast(f32r))
                nc.gpsimd.dma_start(cosT[:], cos_d.ap())
                nc.gpsimd.dma_start(sinT[:], sin_d.ap())
                nc.gpsimd.dma_start(ones[:], ones_d.ap())
                nc.gpsimd.dma_start(masks[:], mask_d.ap())
                nc.gpsimd.dma_start(ident[:], ident_d.ap())

                qt_tiles, kt_tiles, vh_tiles = {}, {}, {}

                def qkv_steps(pair):
                    """QKV projection + RoPE for head pair `pair`.

                    Yields after each 16-matmul PE group; post-ops of group
                    i are emitted after group i+1's matmuls so the PE never
                    waits on the ACT/DVE tail of the previous group.
                    """
                    wts = {}
                    for hh in range(2):
                        h = 2 * pair + hh
                        for tag, wd in (("q", wq_d), ("k", wk_d)):
                            wt = xwp.tile([P, KO, D], bf16, tag="w", bufs=8,
                                          name=f"w{tag}{h}")
                            nc.sync.dma_start(wt[:], wd.ap()[h])
                            wts[(hh, tag)] = wt
                    wvc = xwp.tile([P, KO, 256], bf16, tag="wv", bufs=2)
                    nc.sync.dma_start(wvc[:], wv_d.ap()[pair])
                    qt = qkp.tile([P, 2, T], bf16, tag="qt", name=f"qt{pair}")
                    kt = qkp.tile([P, 2, T], bf16, tag="kt", name=f"kt{pair}")
                    vh = vhp.tile([P, KO, 256], bf16, tag="vh", name=f"vh{pair}")
                    qt_tiles[pair], kt_tiles[pair], vh_tiles[pair] = qt, kt, vh

                    pend = None

                    def emit_post(p):
                        if p[0] == "qk":
                            _, ps, hh, tag, tch = p
                            tsl = slice(tch * 512, (tch + 1) * 512)
                            raw = rp.tile([P, 512], f32r, tag="raw", bufs=2)
                            nc.scalar.copy(raw[:], ps[:])
                            ps2 = psA.tile([P, 512], f32, tag="swap", bufs=1)
                            nc.tensor.matmul(ps2[:], swpT[:], raw[:],
                                             start=True, stop=True)
                            tA = rp.tile([P, 512], f32, tag="tA", bufs=2)
                            nc.vector.tensor_mul(tA[:], raw[:], cosT[:, tsl])
                            tB = rp.tile([P, 512], f32, tag="tB", bufs=2)
                            nc.vector.tensor_mul(tB[:], ps2[:], sinT[:, tsl])
                            dst = qt if tag == "q" else kt
                            nc.vector.tensor_add(dst[:, hh, tsl], tA[:], tB[:])
                        else:
                            _, ps, tch, tb = p
                            nc.scalar.copy(vh[:, tch * 4 + tb, :], ps[:])

                    for tch in range(4):
                        xck = xwp.tile([P, KO, 512], bf16, tag="x", bufs=2)
                        nc.sync.dma_start(xck[:], xq_d.ap()[tch])
                        groups = [("qk", 0, "q"), ("v", 0), ("qk", 0, "k"),
                                  ("v", 1), ("qk", 1, "q"), ("v", 2),
                                  ("qk", 1, "k"), ("v", 3)]
                        for g in groups:
                            if g[0] == "qk":
                                _, hh, tag = g
                                ps = psA.tile([P, 512], f32, tag="qk", bufs=2)
                                wt = wts[(hh, tag)]
                                for ki in range(KO):
                                    nc.tensor.matmul(
                                        ps[:], wt[:, ki, :], xck[:, ki, :],
                                        start=(ki == 0), stop=(ki == KO - 1))
                                newpend = ("qk", ps, hh, tag, tch)
                            else:
                                tb = g[1]
                                ps = psV.tile([P, 256], f32, tag="v", bufs=1)
                                for ki in range(KO):
                                    nc.tensor.matmul(
                                        ps[:], xck[:, ki, tb * P:(tb + 1) * P],
                                        wvc[:, ki, :],
                                        start=(ki == 0), stop=(ki == KO - 1))
                                newpend = ("v", ps, tch, tb)
                            if pend is not None:
                                emit_post(pend)
                            pend = newpend
                            yield
                    emit_post(pend)
                    yield

                def attn_steps(pair):
                    """Attention for head pair `pair` from resident qT/kT/v.

                    AV matmuls trail the score matmuls by 2 steps so the
                    scores->exp->(mask)->AV cross-engine chain never stalls
                    the in-order PE stream.
                    """
                    qt, kt, vh = qt_tiles[pair], kt_tiles[pair], vh_tiles[pair]
                    for hh in range(2):
                        h = 2 * pair + hh
                        csl = slice(hh * 128, (hh + 1) * 128)
                        for qb in range(NQ):
                            nkb = 4 * (qb + 1)
                            qsl = slice(qb * 512, (qb + 1) * 512)
                            ps_o = psO.tile([P, 512], f32, tag="o", bufs=1)
                            ps_n = psN.tile([1, 512], f32, tag="n", bufs=1)
                            avq = []

                            def emit_av(k2, p2):
                                # AV + exact-f32 row-sum accumulation, both
                                # consuming pt at lag-2 behind the scores
                                nc.tensor.matmul(
                                    ps_o[:], vh[:, k2, csl], p2[:],
                                    start=(k2 == 0), stop=(k2 == nkb - 1))
                                nc.tensor.matmul(
                                    ps_n[:], ones[:], p2[:],
                                    start=(k2 == 0), stop=(k2 == nkb - 1))

                            for kb in range(nkb):
                                ps_s = psS.tile([P, 512], f32, tag="s", bufs=2)
                                nc.tensor.matmul(
                                    ps_s[:], kt[:, hh, kb * P:(kb + 1) * P],
                                    qt[:, hh, qsl], start=True, stop=True)
                                pt = ptp.tile([P, 512], bf16, tag="pt", bufs=4)
                                nc.scalar.activation(pt[:], ps_s[:], Exp,
                                                     scale=SCALE)
                                j = kb - 4 * qb
                                if j >= 0:  # diagonal block: causal mask
                                    ptm = ptp.tile([P, 512], bf16, tag="ptm",
                                                   bufs=3)
                                    nc.gpsimd.tensor_mul(ptm[:], pt[:],
                                                         masks[:, j, :])
                                    pt = ptm
                                avq.append((kb, pt))
                                if len(avq) > 2:
                                    emit_av(*avq.pop(0))
                                yield
                            while avq:
                                emit_av(*avq.pop(0))
                                yield
                            inv = smp.tile([1, 512], f32, tag="inv", bufs=2)
                            nc.vector.reciprocal(inv[:], ps_n[:])
                            nc.gpsimd.dma_start(inv_s.ap()[h, qb], inv[:])
                            bcast = smp.tile([P, 512], f32, tag="bc", bufs=2)
                            nc.gpsimd.dma_start(
                                bcast[:],
                                inv_s.ap()[h, qb].to_broadcast((P, 512)))
                            nc.vector.tensor_mul(o_all[:, h, qsl], ps_o[:],
                                                 bcast[:])
                            yield

                with tc.tile_pool(name="xw", bufs=2) as xwp, \
                     tc.tile_pool(name="rope", bufs=2) as rp:
                    ag = None
                    for pair in range(NPAIR):
                        qg = qkv_steps(pair)
                        if zip_pairs:
                            ag = attn_steps(pair - 1) if pair > 0 else None
                            for _ in qg:
                                if ag is not None:
                                    for _k in range(3):
                                        if next(ag, None) is None:
                                            ag = None
                                            break
                            if ag is not None:
                                for _ in ag:
                                    pass
                        else:
                            for _ in qg:
                                pass
                            for _ in attn_steps(pair):
                                pass
                    # attention for the last pair (no QKV left to overlap)
                    if zip_pairs:
                        for _ in attn_steps(NPAIR - 1):
                            pass

                # output projection from resident O^T
                with tc.tile_pool(name="wpp", bufs=2) as wpp, \
                     tc.tile_pool(name="yp", bufs=3) as yp:
                    for co in range(C // 512):
                        wpc = wpp.tile([P, HG, 512], bf16, tag="wpc")
                        nc.sync.dma_start(wpc[:], wp_d.ap()[co])
                        for qc in range(T // P):
                            ps = psA.tile([P, 512], f32, tag="qk")
                            for h in range(HG):
                                nc.tensor.matmul(
                                    ps[:], o_all[:, h, qc * P:(qc + 1) * P],
                                    wpc[:, h, :],
                                    start=(h == 0), stop=(h == HG - 1))
                            ysb = yp.tile([P, 512], f32, tag="ysb")
                            nc.scalar.copy(ysb[:], ps[:])
                            nc.sync.dma_start(
                                y_d.ap()[qc * P:(qc + 1) * P,
                                         co * 512:(co + 1) * 512], ysb[:])
                            if bench_mode and co == C // 512 - 1 and qc == T // P - 1:
                                nc.sync.dma_start(tok_d.ap(), ysb[:, :P])

    nc.finalize()
    return nc


def _host_tables():
    thetas = 1.0 / (ROPE_BASE ** (np.arange(0, D, 2, dtype=np.float32) / D))  # [64]
    t = np.arange(T, dtype=np.float32)
    freqs = t[None, :] * thetas[:, None]                     # [64, T]
    cosT = np.repeat(np.cos(freqs), 2, axis=0).astype(np.float32)  # [128, T]
    sinT = np.repeat(np.sin(freqs), 2, axis=0).astype(np.float32)
    swapT = np.zeros((P, P), np.float32)
    for i in range(0, P, 2):
        swapT[i, i + 1] = 1.0      # (S^T)[2i, 2i+1] = +1
        swapT[i + 1, i] = -1.0     # (S^T)[2i+1, 2i] = -1
    ones = np.ones((P, 1), BF16)
    ki = np.arange(P)[:, None]
    qi = np.arange(512)[None, :]
    masks = np.stack([(ki + 128 * j <= qi).astype(np.float32) for j in range(4)],
                     axis=1)  # [128, 4, 512]
    ident = np.eye(P, dtype=BF16)
    return (cosT, sinT, swapT, ones,
            np.ascontiguousarray(masks).astype(BF16), ident)


class _Runner:
    """Compile the bass program to a PJRT executable once; rerun cheaply.

    Mirrors concourse.bass2jax.run_bass_via_pjrt but caches the jitted
    shard_map callable so repeated kernel() calls (and benchmarking) do not
    pay tracing + compile again.
    """

    def __init__(self, nc):
        import jax
        from jax.sharding import Mesh, PartitionSpec
        try:
            from jax.experimental.shard_map import shard_map
        except ImportError:
            from jax import shard_map
        from concourse import bass2jax, mybir

        bass2jax.install_neuronx_cc_hook()
        self.jax = jax
        self.nc = nc
        assert nc.dbg_addr is None or not nc.dbg_callbacks
        partition_name = (nc.partition_id_tensor.name
                          if nc.partition_id_tensor else None)

        in_names, out_names, out_avals, zero_shapes = [], [], [], []
        for alloc in nc.m.functions[0].allocations:
            if not isinstance(alloc, mybir.MemoryLocationSet):
                continue
            name = alloc.memorylocations[0].name
            if alloc.kind == "ExternalInput":
                if name != partition_name and name != (
                        nc.dbg_addr.name if nc.dbg_addr else None):
                    in_names.append(name)
            elif alloc.kind == "ExternalOutput":
                shape = tuple(alloc.tensor_shape)
                dtype = mybir.dt.np(alloc.dtype)
                out_names.append(name)
                out_avals.append(jax.core.ShapedArray(shape, dtype))
                zero_shapes.append((shape, dtype))
        self.in_names, self.out_names = in_names, out_names
        self.out_avals, self.zero_shapes = out_avals, zero_shapes
        n_params, n_outs = len(in_names), len(out_names)
        self.n_params = n_params

        all_names = list(in_names) + list(out_names)
        if nc.dbg_addr is not None:
            all_names.append(nc.dbg_addr.name)
        if partition_name is not None:
            all_names.append(partition_name)

        def _body(*args):
            operands = list(args)
            if nc.dbg_addr is not None:
                operands.append(jax.numpy.zeros((1, 2), "uint32"))
            if partition_name is not None:
                operands.append(bass2jax.partition_id_tensor())
            outs = bass2jax._bass_exec_p.bind(
                *operands,
                out_avals=tuple(out_avals),
                in_names=tuple(all_names),
                out_names=tuple(out_names),
                lowering_input_output_aliases=(),
                sim_require_finite=True,
                sim_require_nnan=True,
                nc=nc,
            )
            return tuple(outs)

        devices = jax.devices()[:N_CORES]
        self.mesh = Mesh(np.asarray(devices), ("core",))
        self.pspec = PartitionSpec("core")
        in_specs = (self.pspec,) * (n_params + n_outs)
        out_specs = (self.pspec,) * n_outs
        donate = tuple(range(n_params, n_params + n_outs))
        self.fn = jax.jit(
            shard_map(_body, mesh=self.mesh, in_specs=in_specs,
                      out_specs=out_specs, check_rep=False),
            donate_argnums=donate, keep_unused=True)

    def concat_inputs(self, in_maps):
        return [np.concatenate([np.asarray(in_maps[c][n])
                                for c in range(N_CORES)], axis=0)
                for n in self.in_names]

    def device_inputs(self, concat_in):
        from jax.sharding import NamedSharding
        sh = NamedSharding(self.mesh, self.pspec)
        return [self.jax.device_put(a, sh) for a in concat_in]

    def zeros(self, on_device=False):
        zs = [np.zeros((N_CORES * s[0], *s[1:]), d) for s, d in self.zero_shapes]
        if on_device:
            from jax.sharding import NamedSharding
            sh = NamedSharding(self.mesh, self.pspec)
            zs = [self.jax.device_put(z, sh) for z in zs]
        return zs

    def run(self, args):
        out_arrs = self.fn(*args)
        return [
            {n: np.asarray(out_arrs[i]).reshape(N_CORES, *self.out_avals[i].shape)[c]
             for i, n in enumerate(self.out_names)}
            for c in range(N_CORES)
        ]


_runner = None


def _get_runner():
    global _cached, _runner
    if _runner is None:
        if _cached is None:
            _cached = _build_program()
        _runner = _Runner(_cached)
    return _runner


def _make_in_maps(x, W_qkv, W_proj):
    cosT, sinT, swapT, ones, masks, ident = _host_tables()
    in_maps = []
    for c in range(N_CORES):
        b, g = c // G, c % G
        cols = slice(g * CG, (g + 1) * CG)
        xT = x[b].T  # [C, T]
        wq = W_qkv[:, 0 * C:1 * C][:, cols]
        wk = W_qkv[:, 1 * C:2 * C][:, cols]
        wv = W_qkv[:, 2 * C:3 * C][:, cols]
        wpm = W_proj[g * CG:(g + 1) * CG, :]
        in_maps.append({
            # [C, T] -> [tchunk, p, ko, 512]
            "xq": np.ascontiguousarray(
                xT.reshape(KO, P, 4, 512).transpose(2, 1, 0, 3)).astype(BF16),
            # [C, CG] -> [h, p, ko, D]
            "wq": np.ascontiguousarray(
                wq.reshape(KO, P, HG, D).transpose(2, 1, 0, 3)).astype(BF16),
            "wk": np.ascontiguousarray(
                wk.reshape(KO, P, HG, D).transpose(2, 1, 0, 3)).astype(BF16),
            # [C, CG] -> [pair, p, ko, 256]
            "wv": np.ascontiguousarray(
                wv.reshape(KO, P, NPAIR, 256).transpose(2, 1, 0, 3)).astype(BF16),
            # [CG, C] -> [co, p, hb, 512]
            "wp": np.ascontiguousarray(
                wpm.reshape(HG, P, 4, 512).transpose(2, 1, 0, 3)).astype(BF16),
            "cosT": cosT, "sinT": sinT, "swapT": swapT,
            "ones": ones, "masks": masks, "ident": ident,
        })
    return in_maps


def kernel(x, W_qkv, W_proj):
    x = np.asarray(x, dtype=np.float32)
    W_qkv = np.asarray(W_qkv, dtype=np.float32)
    W_proj = np.asarray(W_proj, dtype=np.float32)

    r = _get_runner()
    concat_in = r.concat_inputs(_make_in_maps(x, W_qkv, W_proj))
    results = r.run(concat_in + r.zeros())
    out = np.empty((B, T, C), np.float32)
    for b in range(B):
        out[b] = results[2 * b]["y"] + results[2 * b + 1]["y"]
    return out

